# revision 16
# baseline (speedup 1.0000x reference)
"""Multi-head causal attention (B=2,S=2048,D=1024,H=16,dqk=dv=64) on 8 trn2
NeuronCores.

Sharding: tensor-parallel over heads (2 heads/core) for QKV+attention, then
four pipelined AllToAlls (one per (batch, local-head)) flip to row-parallel
(512 rows/core, interleaved 256-row blocks) for the output projection.

All matmuls run in bf16 (fp32 PSUM accumulation); fp32 is kept for biases,
softmax denominators and the final output.

Per-core pipeline:
  A. x^T is host-pretransposed into a per-partition-contiguous layout; all 8
     s-tiles are prefetched up front. QKV projections Q^T/K^T/V^T = W.T @ x^T
     (bias+cast on DVE). V^T -> V natural key-chunks via DMA XBAR transpose
     (192B-aligned chunk pitch - the XBAR needs 64B-aligned destinations).
  B. flash attention in transposed-score layout: S^T[j,i] blocks, causal
     skip, exp on ACT, post-exp bf16 causal mask multiply, P^T @ [V|1]
     accumulates O^T + softmax denominators. The last 4 projection s-tiles
     are interleaved into the first attention group to keep PE dense.
  C. per (b,lh): normalize via reciprocal + PE broadcast, DMA into the
     AllToAll buffer (Scalar DMA queue - the Sync queue is left to the
     collectives, which block it while in flight), fire the chunked A2A.
     A tiny warmup AllToAll absorbs the first-collective penalty.
  D. after both heads of a batch land: out = G @ Wo + bo for this core's
     256-row block of that batch (K=128 packed across both heads).
Host: reassemble the 8 cores' interleaved [2x256,1024] row blocks.
"""

import ml_dtypes
import numpy as np

import bass_rust
import concourse.bass as bass
import concourse.mybir as mybir
import concourse.tile as tile
from concourse import bass_utils
from concourse.vector_clock import ScopedClock

# ---------------------------------------------------------------------------
# Workaround for this container's walrus build: it accepts at most ONE sync
# wait per instruction, but Tile emits several (tail drain + stage-1B waits).
# Split extra waits onto same-engine NoOps placed right before the instruction.
# ---------------------------------------------------------------------------

_waitsplit_cnt = [0]


def _patched_drain_and_barrier(self, tick_clock, wait_clock):
    nc = self.nc
    drain_inst = nc.sync.drain()
    wait_clock.add_sem_waits(
        drain_inst.ins, ScopedClock({None: tick_clock.global_clock})
    )
    si = drain_inst.ins.sync_info
    waits = list(si.on_wait) if si is not None else []
    if len(waits) > 1:
        drain_inst.ins.sync_info = bass_rust.SyncInfo(
            on_wait=[waits[0]], on_update=list(si.on_update)
        )
        for w in waits[1:]:
            d2 = nc.sync.drain()
            d2.ins.sync_info = bass_rust.SyncInfo(on_wait=[w], on_update=[])
    nc.all_engine_barrier()
    popped = nc._tile_sem_poison_stack.pop()
    assert popped is self._sem_poison
    nc.clear_and_free_semaphores(list(self.sems.allocated().values()))
    nc.all_engine_barrier()


tile.TileContext._drain_and_barrier = _patched_drain_and_barrier


def _split_multi_waits(nc):
    for f in nc.m.functions:
        for bb in f.blocks:
            insts = bb.instructions
            out = []
            dirty = False
            for inst in insts:
                si = inst.sync_info
                if si is not None and len(si.on_wait) > 1:
                    waits = list(si.on_wait)
                    for w in waits[:-1]:
                        nop = mybir.InstNoOp(
                            name=f"waitsplit_{_waitsplit_cnt[0]}", ins=[], outs=[]
                        )
                        _waitsplit_cnt[0] += 1
                        nop.engine = inst.engine
                        nop.sync_info = bass_rust.SyncInfo(on_wait=[w], on_update=[])
                        out.append(nop)
                    inst.sync_info = bass_rust.SyncInfo(
                        on_wait=[waits[-1]], on_update=list(si.on_update)
                    )
                    dirty = True
                out.append(inst)
            if dirty:
                bb.instructions = out


# ---------------------------------------------------------------------------
# Problem constants (hardcoded, self-contained)
# ---------------------------------------------------------------------------
B, S, D = 2, 2048, 1024
H, E = 16, 64           # heads, head dim
NCORES = 8
HL = H // NCORES        # heads per core = 2
BS = B * S              # 4096 flattened rows
ND = D // 128           # 8 d-chunks
ST = 512                # projection s-tile (rhs cols)
NST = BS // ST          # 8
TI = 512                # attention i-tile
NT_I = S // TI          # 4 per batch
TJ = 128                # key chunk
NJC = S // TJ           # 16 per batch
RQ = 256                # rows per core per batch (interleaved sharding)
VP = 96                 # vsb chunk pitch (192B, 64B-aligned for the XBAR)

f32 = mybir.dt.float32
bf16 = mybir.dt.bfloat16
Exp = mybir.ActivationFunctionType.Exp
bfdt = np.dtype(ml_dtypes.bfloat16)

_built = [None]


def _build():
    nc = bass.Bass("TRN2", target_bir_lowering=False, debug=False,
                   num_devices=NCORES)

    xt_d = nc.dram_tensor("xt", (128, NST, ND, ST), bf16,
                          kind="ExternalInput").ap()
    wq_d = nc.dram_tensor("wq", (D, 128), bf16, kind="ExternalInput").ap()
    wk_d = nc.dram_tensor("wk", (D, 128), bf16, kind="ExternalInput").ap()
    wv_d = nc.dram_tensor("wv", (D, 128), bf16, kind="ExternalInput").ap()
    bq_d = nc.dram_tensor("bq", (128, 1), f32, kind="ExternalInput").ap()
    bk_d = nc.dram_tensor("bk", (128, 1), f32, kind="ExternalInput").ap()
    bv_d = nc.dram_tensor("bv", (128, 1), f32, kind="ExternalInput").ap()
    wo_d = nc.dram_tensor("wo", (D, D), bf16, kind="ExternalInput").ap()
    bob_d = nc.dram_tensor("bob", (128, D), f32, kind="ExternalInput").ap()
    maskb_d = nc.dram_tensor("maskb", (128, 128), bf16,
                             kind="ExternalInput").ap()
    sel0_d = nc.dram_tensor("sel0", (128, E), bf16,
                            kind="ExternalInput").ap()

    out_d = nc.dram_tensor("out", (2 * RQ, D), f32, kind="ExternalOutput").ap()

    # one AllToAll per (batch, local head): quarter-sized, pipelined
    a2a_in = [[nc.dram_tensor(f"a2a_in{b}_{lh}", (NCORES, E, RQ), bf16,
                              kind="Internal").ap() for lh in range(HL)]
              for b in range(B)]
    a2a_out = [[nc.dram_tensor(f"a2a_out{b}_{lh}", (NCORES, E, RQ), bf16,
                               kind="Internal").ap() for lh in range(HL)]
               for b in range(B)]

    with tile.TileContext(nc) as tc:
        with tc.tile_pool(name="persist", bufs=1) as pp:
            # activations, feature-on-partition, [2 heads x 64, B*S]
            qt = pp.tile([128, BS], bf16, tag="qt")
            kt = pp.tile([128, BS], bf16, tag="kt")
            vt = pp.tile([128, BS], bf16, tag="vt")
            wq_sb = pp.tile([128, ND, 128], bf16, tag="wq")
            wk_sb = pp.tile([128, ND, 128], bf16, tag="wk")
            wv_sb = pp.tile([128, ND, 128], bf16, tag="wv")
            wo_sb = pp.tile([128, ND, D], bf16, tag="wo")
            bq_sb = pp.tile([128, 1], f32, tag="bq")
            bk_sb = pp.tile([128, 1], f32, tag="bk")
            bv_sb = pp.tile([128, 1], f32, tag="bv")
            bob_sb = pp.tile([128, D], f32, tag="bob")
            maskb_sb = pp.tile([128, 128], bf16, tag="maskb")
            sel0_sb = pp.tile([128, E], bf16, tag="sel0")
            # V natural chunks; col 64 of each 96-elem chunk holds the ones
            # for the denominator trick (whole tile memset to 1.0 first)
            vsb = [pp.tile([128, NJC, VP], bf16, tag=f"vsb{i}",
                           name=f"vsb{i}")
                   for i in range(B * HL)]
            # staged AllToAll results, per batch: [128 feat, 8 srccore, 256]
            gsb = [pp.tile([128, NCORES, RQ], bf16, tag=f"gs{b}",
                           name=f"gs{b}") for b in range(B)]

            nc.scalar.dma_start(wq_sb[:], wq_d.rearrange("(c p) e -> p c e",
                                                       p=128))
            nc.scalar.dma_start(wk_sb[:], wk_d.rearrange("(c p) e -> p c e",
                                                       p=128))
            nc.scalar.dma_start(wv_sb[:], wv_d.rearrange("(c p) e -> p c e",
                                                       p=128))
            nc.scalar.dma_start(bq_sb[:], bq_d[:])
            nc.scalar.dma_start(bk_sb[:], bk_d[:])
            nc.scalar.dma_start(bv_sb[:], bv_d[:])
            nc.scalar.dma_start(maskb_sb[:], maskb_d[:])
            nc.scalar.dma_start(sel0_sb[:], sel0_d[:])
            for i in range(B * HL):
                with nc.allow_low_precision(reason="bf16 ones col"):
                    nc.gpsimd.memset(vsb[i][:], 1.0)

            with tc.tile_pool(name="xts", bufs=NST) as xts_pool, \
                 tc.tile_pool(name="expp", bufs=4) as expp, \
                 tc.tile_pool(name="osbp", bufs=8) as osbp, \
                 tc.tile_pool(name="sepi", bufs=2) as sepi, \
                 tc.tile_pool(name="ob", bufs=3) as ob_pool, \
                 tc.tile_pool(name="ps2", bufs=2, space="PSUM") as ps2_pool, \
                 tc.tile_pool(name="psd", bufs=2, space="PSUM") as psd_pool, \
                 tc.tile_pool(name="po", bufs=2, space="PSUM") as po_pool:

                # prefetch every x^T s-tile (8KB contiguous per partition)
                xts = []
                for st in range(NST):
                    xt_t = xts_pool.tile([128, ND, ST], bf16, tag="xt",
                                         name=f"xts{st}")
                    nc.sync.dma_start(xt_t[:], xt_d[:, st, :, :])
                    xts.append(xt_t)
                    if st == 0:
                        # big late-use weights on the ACT hwdge queue
                        nc.scalar.dma_start(
                            wo_sb[:],
                            wo_d.rearrange("(c p) o -> p c o", p=128))
                        nc.scalar.dma_start(bob_sb[:], bob_d[:])

                def phase_a(st):
                    for wsb, bsb, dst in ((wq_sb, bq_sb, qt),
                                          (wk_sb, bk_sb, kt),
                                          (wv_sb, bv_sb, vt)):
                        ps = psd_pool.tile([128, ST], f32, tag="sd")
                        for dc in range(ND):
                            nc.tensor.matmul(ps[:], wsb[:, dc, :],
                                             xts[st][:, dc, :],
                                             start=(dc == 0),
                                             stop=(dc == ND - 1))
                        with nc.allow_low_precision(reason="bf16 proj"):
                            nc.vector.tensor_scalar_add(
                                dst[:, st * ST:(st + 1) * ST], ps[:], bsb[:])
                    bb_, jc0 = st // 4, 4 * (st % 4)
                    for lh in range(HL):
                        v_t = vsb[bb_ * HL + lh]
                        nc.sync.dma_start(
                            v_t[:, jc0:jc0 + 4, 0:E],
                            vt[lh * E:(lh + 1) * E,
                               bb_ * S + jc0 * TJ: bb_ * S + (jc0 + 4) * TJ],
                            transpose=True)

                def scores_mm(ps_ap, lh, b, jc, t, ncols, coff):
                    nc.tensor.matmul(
                        ps_ap,
                        kt[E * lh:E * (lh + 1),
                           b * S + jc * TJ: b * S + (jc + 1) * TJ],
                        qt[E * lh:E * (lh + 1),
                           b * S + t * TI + coff: b * S + t * TI + coff + ncols],
                        start=True, stop=True)


                def attn_block(b, lh, t):
                    vv = vsb[b * HL + lh]
                    po = po_pool.tile([E + 1, TI], f32, tag="o",
                                      name=f"po{b}_{t}_{lh}")
                    # paired full blocks (jc < 4t)
                    for jp in range(2 * t):
                        jc = 2 * jp
                        ps2 = ps2_pool.tile([128, 2 * TI], f32, tag="s2")
                        scores_mm(ps2[:, 0:TI], lh, b, jc, t, TI, 0)
                        scores_mm(ps2[:, TI:2 * TI], lh, b, jc + 1, t, TI, 0)
                        es = expp.tile([128, 2 * TI], bf16, tag="e")
                        nc.scalar.activation(es[:], ps2[:], Exp, scale=0.125)
                        nc.tensor.matmul(po[:], vv[:, jc, 0:E + 1],
                                         es[:, 0:TI],
                                         start=(jc == 0), stop=False)
                        nc.tensor.matmul(po[:], vv[:, jc + 1, 0:E + 1],
                                         es[:, TI:2 * TI],
                                         start=False, stop=False)
                    # diagonal blocks (ri = 0..3), column-shrunk; causal mask
                    # applied as a post-exp bf16 multiply (cheaper on DVE)
                    for ri in range(4):
                        jc = 4 * t + ri
                        ncols = TI - 128 * ri
                        psd = psd_pool.tile([128, TI], f32, tag="sd")
                        scores_mm(psd[:, 0:ncols], lh, b, jc, t, ncols,
                                  128 * ri)
                        esd = expp.tile([128, TI], bf16, tag="ed")
                        nc.scalar.activation(esd[:, 0:ncols],
                                             psd[:, 0:ncols], Exp,
                                             scale=0.125)
                        with nc.allow_low_precision(reason="bf16 mask"):
                            nc.vector.tensor_mul(esd[:, 0:128],
                                                 esd[:, 0:128], maskb_sb[:])
                        nc.tensor.matmul(
                            po[:, 128 * ri:TI], vv[:, jc, 0:E + 1],
                            esd[:, 0:ncols],
                            start=(jc == 0), stop=(ri == 3))
                    # free the PSUM accumulator, then normalize this
                    # tile right away (streaming epilogue)
                    osb = osbp.tile([E, TI], bf16, tag="osb",
                                    name=f"osb{b}_{t}_{lh}")
                    with nc.allow_low_precision(reason="bf16 O"):
                        nc.vector.tensor_copy(osb[:], po[0:E, :])
                    dnt = sepi.tile([1, TI], f32, tag="dn",
                                    name=f"dn{b}_{lh}_{t}")
                    nc.vector.tensor_copy(dnt[:], po[E:E + 1, :])
                    rec = sepi.tile([1, TI], f32, tag="rec",
                                    name=f"rec{b}_{lh}_{t}")
                    with nc.allow_low_precision(reason="softmax denom"):
                        nc.vector.reciprocal(rec[:], dnt[:])
                    recb = sepi.tile([1, TI], bf16, tag="recb",
                                     name=f"recb{b}_{lh}_{t}")
                    with nc.allow_low_precision(reason="bf16 recip"):
                        nc.vector.tensor_copy(recb[:], rec[:])
                    pb = psd_pool.tile([E, TI], f32, tag="sd")
                    nc.tensor.matmul(pb[:], sel0_sb[0:1, 0:E], recb[:],
                                     start=True, stop=True)
                    ost = sepi.tile([E, TI], bf16, tag="ost",
                                    name=f"ost{b}_{lh}_{t}")
                    with nc.allow_low_precision(reason="bf16 O"):
                        nc.vector.tensor_mul(ost[:], osb[:], pb[:])
                    for hf in range(2):
                        nc.scalar.dma_start(
                            a2a_in[b][lh][2 * t + hf, :, :],
                            ost[:, hf * RQ:(hf + 1) * RQ])

                def epilogue(b, lh):
                    nc.gpsimd.collective_compute(
                        "AllToAll", mybir.AluOpType.bypass,
                        replica_groups=[list(range(NCORES))],
                        ins=[a2a_in[b][lh][:]], outs=[a2a_out[b][lh][:]])

                def stage_gs(b, lh):
                    nc.scalar.dma_start(
                        gsb[b][lh * E:(lh + 1) * E, :, :],
                        a2a_out[b][lh].rearrange("s f r -> f s r"))

                def wo_block(b):
                    for rb in range(RQ // 128):
                        for ot in range(D // 512):
                            pw = psd_pool.tile([128, 512], f32, tag="sd")
                            for fi in range(NCORES):
                                nc.tensor.matmul(
                                    pw[:],
                                    gsb[b][:, fi, rb * 128:(rb + 1) * 128],
                                    wo_sb[:, fi, ot * 512:(ot + 1) * 512],
                                    start=(fi == 0), stop=(fi == NCORES - 1))
                            ob = ob_pool.tile([128, 512], f32, tag="ob")
                            nc.vector.tensor_add(
                                ob[:], pw[:],
                                bob_sb[:, ot * 512:(ot + 1) * 512])
                            nc.scalar.dma_start(
                                out_d[b * RQ + rb * 128:
                                      b * RQ + (rb + 1) * 128,
                                      ot * 512:(ot + 1) * 512],
                                ob[:])

                # batch 0 head 0, with the last 4 projection s-tiles
                # interleaved to keep the PE stream dense
                for st in range(4):
                    phase_a(st)
                for t in range(NT_I):
                    attn_block(0, 0, t)
                    phase_a(4 + t)
                epilogue(0, 0)
                for t in range(NT_I):
                    attn_block(0, 1, t)
                epilogue(0, 1)
                for t in range(NT_I):
                    attn_block(1, 0, t)
                epilogue(1, 0)
                stage_gs(0, 0)
                stage_gs(0, 1)
                wo_block(0)
                stage_gs(1, 0)
                for t in range(NT_I):
                    attn_block(1, 1, t)
                epilogue(1, 1)
                stage_gs(1, 1)
                wo_block(1)

    _split_multi_waits(nc)
    return nc


def _get_nc():
    if _built[0] is None:
        _built[0] = _build()
    return _built[0]


def _host_inputs(x, Wq, bq, Wk, bk, Wv, bv, Wo, bo):
    xf = np.asarray(x, dtype=np.float32).reshape(BS, D)
    # [p, st, c, s] so each s-tile load is 8KB contiguous per partition
    xt = np.ascontiguousarray(
        xf.reshape(NST, ST, ND, 128).transpose(3, 0, 2, 1)).astype(bfdt)
    Wq = np.asarray(Wq, dtype=np.float32)
    Wk = np.asarray(Wk, dtype=np.float32)
    Wv = np.asarray(Wv, dtype=np.float32)
    bq = np.asarray(bq, dtype=np.float32)
    bk = np.asarray(bk, dtype=np.float32)
    bv = np.asarray(bv, dtype=np.float32)
    Wo = np.ascontiguousarray(np.asarray(Wo, dtype=np.float32)).astype(bfdt)
    bo = np.asarray(bo, dtype=np.float32)

    jj = np.arange(128, dtype=np.int64)[:, None]
    ii = np.arange(128, dtype=np.int64)[None, :]
    maskb = np.where(jj <= ii, 1.0, 0.0).astype(bfdt)
    bob = np.tile(bo[None, :], (128, 1)).astype(np.float32)
    sel0 = np.zeros((128, E), dtype=np.float32)
    sel0[0, :] = 1.0
    sel0 = sel0.astype(bfdt)

    in_maps = []
    for c in range(NCORES):
        hs = slice(HL * c, HL * (c + 1))
        in_maps.append({
            "xt": xt,
            "wq": np.ascontiguousarray(
                Wq[hs].transpose(1, 0, 2).reshape(D, 128)).astype(bfdt),
            "wk": np.ascontiguousarray(
                Wk[hs].transpose(1, 0, 2).reshape(D, 128)).astype(bfdt),
            "wv": np.ascontiguousarray(
                Wv[hs].transpose(1, 0, 2).reshape(D, 128)).astype(bfdt),
            "bq": np.ascontiguousarray(bq[hs].reshape(128, 1)),
            "bk": np.ascontiguousarray(bk[hs].reshape(128, 1)),
            "bv": np.ascontiguousarray(bv[hs].reshape(128, 1)),
            "wo": Wo,
            "bob": bob,
            "maskb": maskb,
            "sel0": sel0,
        })
    return in_maps


def kernel(x, Wq, bq, Wk, bk, Wv, bv, Wo, bo, _trace=False, _tmpdir=None):
    nc = _get_nc()
    in_maps = _host_inputs(x, Wq, bq, Wk, bk, Wv, bv, Wo, bo)
    res = bass_utils.run_bass_kernel_spmd(
        nc, in_maps, core_ids=list(range(NCORES)),
        trace=_trace, tmpdir=_tmpdir)
    out = np.empty((BS, D), dtype=np.float32)
    for c in range(NCORES):
        r = res.results[c]["out"]
        out[RQ * c: RQ * (c + 1)] = r[0:RQ]
        out[S + RQ * c: S + RQ * (c + 1)] = r[RQ:2 * RQ]
    kernel.last_exec_time_ns = res.exec_time_ns
    kernel.last_results = res
    return out.reshape(B, S, D)


kernel.last_exec_time_ns = None
kernel.last_results = None


# revision 17
# speedup vs baseline: 1.1903x; 1.1903x over previous
"""Multi-head causal attention (B=2,S=2048,D=1024,H=16,dqk=dv=64) on 8 trn2
NeuronCores.

Sharding: tensor-parallel over heads (2 heads/core) for QKV+attention, then
four pipelined AllToAlls (one per (batch, local-head)) flip to row-parallel
(512 rows/core, interleaved 256-row blocks) for the output projection.

All matmuls run in bf16 (fp32 PSUM accumulation); fp32 is kept for biases,
softmax denominators and the final output.

Per-core pipeline:
  A. x^T is host-pretransposed into a per-partition-contiguous layout; all 8
     s-tiles are prefetched up front. QKV projections Q^T/K^T/V^T = W.T @ x^T
     (bias+cast on DVE). V^T -> V natural key-chunks via DMA XBAR transpose
     (192B-aligned chunk pitch - the XBAR needs 64B-aligned destinations).
  B. flash attention in transposed-score layout: S^T[j,i] blocks, causal
     skip, exp on ACT, post-exp bf16 causal mask multiply, P^T @ [V|1]
     accumulates O^T + softmax denominators. The last 4 projection s-tiles
     are interleaved into the first attention group to keep PE dense.
  C. per (b,lh): normalize via reciprocal + PE broadcast, DMA into the
     AllToAll buffer (Scalar DMA queue - the Sync queue is left to the
     collectives, which block it while in flight), fire the chunked A2A.
     A tiny warmup AllToAll absorbs the first-collective penalty.
  D. after both heads of a batch land: out = G @ Wo + bo for this core's
     256-row block of that batch (K=128 packed across both heads).
Host: reassemble the 8 cores' interleaved [2x256,1024] row blocks.
"""

import ml_dtypes
import numpy as np

import bass_rust
import concourse.bass as bass
import concourse.mybir as mybir
import concourse.tile as tile
from concourse import bass_utils
from concourse.vector_clock import ScopedClock

# ---------------------------------------------------------------------------
# Workaround for this container's walrus build: it accepts at most ONE sync
# wait per instruction, but Tile emits several (tail drain + stage-1B waits).
# Split extra waits onto same-engine NoOps placed right before the instruction.
# ---------------------------------------------------------------------------

_waitsplit_cnt = [0]


def _patched_drain_and_barrier(self, tick_clock, wait_clock):
    nc = self.nc
    drain_inst = nc.sync.drain()
    wait_clock.add_sem_waits(
        drain_inst.ins, ScopedClock({None: tick_clock.global_clock})
    )
    si = drain_inst.ins.sync_info
    waits = list(si.on_wait) if si is not None else []
    if len(waits) > 1:
        drain_inst.ins.sync_info = bass_rust.SyncInfo(
            on_wait=[waits[0]], on_update=list(si.on_update)
        )
        for w in waits[1:]:
            d2 = nc.sync.drain()
            d2.ins.sync_info = bass_rust.SyncInfo(on_wait=[w], on_update=[])
    nc.all_engine_barrier()
    popped = nc._tile_sem_poison_stack.pop()
    assert popped is self._sem_poison
    nc.clear_and_free_semaphores(list(self.sems.allocated().values()))
    nc.all_engine_barrier()


tile.TileContext._drain_and_barrier = _patched_drain_and_barrier


def _split_multi_waits(nc):
    for f in nc.m.functions:
        for bb in f.blocks:
            insts = bb.instructions
            out = []
            dirty = False
            for inst in insts:
                si = inst.sync_info
                if si is not None and len(si.on_wait) > 1:
                    waits = list(si.on_wait)
                    for w in waits[:-1]:
                        nop = mybir.InstNoOp(
                            name=f"waitsplit_{_waitsplit_cnt[0]}", ins=[], outs=[]
                        )
                        _waitsplit_cnt[0] += 1
                        nop.engine = inst.engine
                        nop.sync_info = bass_rust.SyncInfo(on_wait=[w], on_update=[])
                        out.append(nop)
                    inst.sync_info = bass_rust.SyncInfo(
                        on_wait=[waits[-1]], on_update=list(si.on_update)
                    )
                    dirty = True
                out.append(inst)
            if dirty:
                bb.instructions = out


# ---------------------------------------------------------------------------
# Problem constants (hardcoded, self-contained)
# ---------------------------------------------------------------------------
B, S, D = 2, 2048, 1024
H, E = 16, 64           # heads, head dim
NCORES = 8
HL = H // NCORES        # heads per core = 2
BS = B * S              # 4096 flattened rows
ND = D // 128           # 8 d-chunks
ST = 512                # projection s-tile (rhs cols)
NST = BS // ST          # 8
TI = 512                # attention i-tile
NT_I = S // TI          # 4 per batch
TJ = 128                # key chunk
NJC = S // TJ           # 16 per batch
RQ = 256                # rows per core per batch (interleaved sharding)
VP = 96                 # vsb chunk pitch (192B, 64B-aligned for the XBAR)

f32 = mybir.dt.float32
bf16 = mybir.dt.bfloat16
Exp = mybir.ActivationFunctionType.Exp
bfdt = np.dtype(ml_dtypes.bfloat16)

_built = [None]


def _build():
    nc = bass.Bass("TRN2", target_bir_lowering=False, debug=False,
                   num_devices=NCORES)

    xt_d = nc.dram_tensor("xt", (128, NST, ND, ST), bf16,
                          kind="ExternalInput").ap()
    wq_d = nc.dram_tensor("wq", (D, 128), bf16, kind="ExternalInput").ap()
    wk_d = nc.dram_tensor("wk", (D, 128), bf16, kind="ExternalInput").ap()
    wv_d = nc.dram_tensor("wv", (D, 128), bf16, kind="ExternalInput").ap()
    bq_d = nc.dram_tensor("bq", (128, 1), f32, kind="ExternalInput").ap()
    bk_d = nc.dram_tensor("bk", (128, 1), f32, kind="ExternalInput").ap()
    bv_d = nc.dram_tensor("bv", (128, 1), f32, kind="ExternalInput").ap()
    wo_d = nc.dram_tensor("wo", (D, D), bf16, kind="ExternalInput").ap()
    bob_d = nc.dram_tensor("bob", (128, D), f32, kind="ExternalInput").ap()
    maskb_d = nc.dram_tensor("maskb", (128, 128), bf16,
                             kind="ExternalInput").ap()
    sel32_d = nc.dram_tensor("sel32", (128, 4 * E), bf16,
                             kind="ExternalInput").ap()

    out_d = nc.dram_tensor("out", (2 * RQ, D), f32, kind="ExternalOutput").ap()
    warm_d = nc.dram_tensor("warm", (NCORES, 64), bf16, kind="Internal").ap()
    warm_o = nc.dram_tensor("warm_o", (NCORES, 64), bf16,
                            kind="Internal").ap()

    # one AllToAll per (batch, local head): quarter-sized, pipelined
    a2a_in = [[nc.dram_tensor(f"a2a_in{b}_{lh}", (NCORES, E, RQ), bf16,
                              kind="Internal").ap() for lh in range(HL)]
              for b in range(B)]
    a2a_out = [[nc.dram_tensor(f"a2a_out{b}_{lh}", (NCORES, E, RQ), bf16,
                               kind="Internal").ap() for lh in range(HL)]
               for b in range(B)]

    with tile.TileContext(nc) as tc:
        with tc.tile_pool(name="persist", bufs=1) as pp:
            # activations, feature-on-partition, [2 heads x 64, B*S]
            qt = pp.tile([128, BS], bf16, tag="qt")
            kt = pp.tile([128, BS], bf16, tag="kt")
            vt = pp.tile([128, BS], bf16, tag="vt")
            wq_sb = pp.tile([128, ND, 128], bf16, tag="wq")
            wk_sb = pp.tile([128, ND, 128], bf16, tag="wk")
            wv_sb = pp.tile([128, ND, 128], bf16, tag="wv")
            wo_sb = pp.tile([128, ND, D], bf16, tag="wo")
            bq_sb = pp.tile([128, 1], f32, tag="bq")
            bk_sb = pp.tile([128, 1], f32, tag="bk")
            bv_sb = pp.tile([128, 1], f32, tag="bv")
            bob_sb = pp.tile([128, D], f32, tag="bob")
            maskb_sb = pp.tile([128, 128], bf16, tag="maskb")
            sel32_sb = pp.tile([128, 4 * E], bf16, tag="sel32")
            # V natural chunks; col 64 of each 96-elem chunk holds the ones
            # for the denominator trick (whole tile memset to 1.0 first)
            vsb = [pp.tile([128, NJC, VP], bf16, tag=f"vsb{i}",
                           name=f"vsb{i}")
                   for i in range(B * HL)]
            # staged AllToAll results, per batch: [128 feat, 8 srccore, 256]
            gsb = [pp.tile([128, NCORES, RQ], bf16, tag=f"gs{b}",
                           name=f"gs{b}") for b in range(B)]

            nc.scalar.dma_start(wq_sb[:], wq_d.rearrange("(c p) e -> p c e",
                                                       p=128))
            nc.scalar.dma_start(wk_sb[:], wk_d.rearrange("(c p) e -> p c e",
                                                       p=128))
            nc.scalar.dma_start(wv_sb[:], wv_d.rearrange("(c p) e -> p c e",
                                                       p=128))
            nc.scalar.dma_start(bq_sb[:], bq_d[:])
            nc.scalar.dma_start(bk_sb[:], bk_d[:])
            nc.scalar.dma_start(bv_sb[:], bv_d[:])
            nc.scalar.dma_start(maskb_sb[:], maskb_d[:])
            nc.scalar.dma_start(sel32_sb[:], sel32_d[:])
            for i in range(B * HL):
                with nc.allow_low_precision(reason="bf16 ones col"):
                    nc.gpsimd.memset(vsb[i][:], 1.0)

            with tc.tile_pool(name="xts", bufs=NST) as xts_pool, \
                 tc.tile_pool(name="expp", bufs=4) as expp, \
                 tc.tile_pool(name="osbp", bufs=8) as osbp, \
                 tc.tile_pool(name="sepi", bufs=2) as sepi, \
                 tc.tile_pool(name="ob", bufs=3) as ob_pool, \
                 tc.tile_pool(name="ps2", bufs=2, space="PSUM") as ps2_pool, \
                 tc.tile_pool(name="psd", bufs=2, space="PSUM") as psd_pool, \
                 tc.tile_pool(name="po", bufs=2, space="PSUM") as po_pool:

                # prefetch every x^T s-tile (8KB contiguous per partition)
                xts = []
                for st in range(NST):
                    xt_t = xts_pool.tile([128, ND, ST], bf16, tag="xt",
                                         name=f"xts{st}")
                    nc.sync.dma_start(xt_t[:], xt_d[:, st, :, :])
                    xts.append(xt_t)
                    if st == 0:
                        # big late-use weights on the ACT hwdge queue
                        nc.scalar.dma_start(
                            wo_sb[:],
                            wo_d.rearrange("(c p) o -> p c o", p=128))
                        nc.scalar.dma_start(bob_sb[:], bob_d[:])

                def phase_a(st):
                    for wsb, bsb, dst in ((wq_sb, bq_sb, qt),
                                          (wk_sb, bk_sb, kt),
                                          (wv_sb, bv_sb, vt)):
                        ps = psd_pool.tile([128, ST], f32, tag="sd")
                        for dc in range(ND):
                            nc.tensor.matmul(ps[:], wsb[:, dc, :],
                                             xts[st][:, dc, :],
                                             start=(dc == 0),
                                             stop=(dc == ND - 1))
                        with nc.allow_low_precision(reason="bf16 proj"):
                            nc.vector.tensor_scalar_add(
                                dst[:, st * ST:(st + 1) * ST], ps[:], bsb[:])
                    bb_, jc0 = st // 4, 4 * (st % 4)
                    for lh in range(HL):
                        v_t = vsb[bb_ * HL + lh]
                        nc.sync.dma_start(
                            v_t[:, jc0:jc0 + 4, 0:E],
                            vt[lh * E:(lh + 1) * E,
                               bb_ * S + jc0 * TJ: bb_ * S + (jc0 + 4) * TJ],
                            transpose=True)

                def scores_mm(ps_ap, lh, b, jc, t, ncols, coff):
                    nc.tensor.matmul(
                        ps_ap,
                        kt[E * lh:E * (lh + 1),
                           b * S + jc * TJ: b * S + (jc + 1) * TJ],
                        qt[E * lh:E * (lh + 1),
                           b * S + t * TI + coff: b * S + t * TI + coff + ncols],
                        start=True, stop=True)


                def attn_block(grp, b, lh, t):
                    vv = vsb[b * HL + lh]
                    po = po_pool.tile([E + 1, TI], f32, tag="o",
                                      name=f"po{b}_{t}_{lh}")
                    # paired full blocks (jc < 4t)
                    for jp in range(2 * t):
                        jc = 2 * jp
                        ps2 = ps2_pool.tile([128, 2 * TI], f32, tag="s2")
                        scores_mm(ps2[:, 0:TI], lh, b, jc, t, TI, 0)
                        scores_mm(ps2[:, TI:2 * TI], lh, b, jc + 1, t, TI, 0)
                        es = expp.tile([128, 2 * TI], bf16, tag="e")
                        nc.scalar.activation(es[:], ps2[:], Exp, scale=0.125)
                        nc.tensor.matmul(po[:], vv[:, jc, 0:E + 1],
                                         es[:, 0:TI],
                                         start=(jc == 0), stop=False)
                        nc.tensor.matmul(po[:], vv[:, jc + 1, 0:E + 1],
                                         es[:, TI:2 * TI],
                                         start=False, stop=False)
                    # diagonal blocks (ri = 0..3), column-shrunk; causal mask
                    # applied as a post-exp bf16 multiply (cheaper on DVE)
                    for ri in range(4):
                        jc = 4 * t + ri
                        ncols = TI - 128 * ri
                        psd = psd_pool.tile([128, TI], f32, tag="sd")
                        scores_mm(psd[:, 0:ncols], lh, b, jc, t, ncols,
                                  128 * ri)
                        esd = expp.tile([128, TI], bf16, tag="ed")
                        nc.scalar.activation(esd[:, 0:ncols],
                                             psd[:, 0:ncols], Exp,
                                             scale=0.125)
                        with nc.allow_low_precision(reason="bf16 mask"):
                            nc.vector.tensor_mul(esd[:, 0:128],
                                                 esd[:, 0:128], maskb_sb[:])
                        nc.tensor.matmul(
                            po[:, 128 * ri:TI], vv[:, jc, 0:E + 1],
                            esd[:, 0:ncols],
                            start=(jc == 0), stop=(ri == 3))
                    # free the PSUM accumulator: features + denom row
                    osb = osbp.tile([E, TI], bf16, tag="osb",
                                    name=f"osb{b}_{t}_{lh}")
                    with nc.allow_low_precision(reason="bf16 O"):
                        nc.vector.tensor_copy(osb[:], po[0:E, :])
                    nc.vector.tensor_copy(grp["dng"][32 * t:32 * t + 1, :],
                                          po[E:E + 1, :])
                    grp["osbs"][t] = osb

                def new_grp(b, lh):
                    dng = sepi.tile([128, TI], f32, tag="dn",
                                    name=f"dn{b}_{lh}")
                    nc.gpsimd.memset(dng[:], 1.0)
                    return {"dng": dng, "osbs": [None] * NT_I}

                def epilogue(grp, b, lh):
                    recg = sepi.tile([128, TI], f32, tag="rec",
                                     name=f"rec{b}_{lh}")
                    with nc.allow_low_precision(reason="softmax denom"):
                        nc.vector.reciprocal(recg[:], grp["dng"][:])
                    recb = sepi.tile([128, TI], bf16, tag="recb",
                                     name=f"recb{b}_{lh}")
                    with nc.allow_low_precision(reason="bf16 recip"):
                        nc.vector.tensor_copy(recb[:], recg[:])
                    for t in range(NT_I):
                        pb = psd_pool.tile([E, TI], f32, tag="sd")
                        nc.tensor.matmul(pb[:],
                                         sel32_sb[:, t * E:(t + 1) * E],
                                         recb[:], start=True, stop=True)
                        ost = sepi.tile([E, TI], bf16, tag="ost",
                                        name=f"ost{b}_{lh}_{t}")
                        with nc.allow_low_precision(reason="bf16 O"):
                            nc.vector.tensor_mul(ost[:], grp["osbs"][t][:],
                                                 pb[:])
                        for hf in range(2):
                            nc.scalar.dma_start(
                                a2a_in[b][lh][2 * t + hf, :, :],
                                ost[:, hf * RQ:(hf + 1) * RQ])
                    nc.gpsimd.collective_compute(
                        "AllToAll", mybir.AluOpType.bypass,
                        replica_groups=[list(range(NCORES))],
                        ins=[a2a_in[b][lh][:]], outs=[a2a_out[b][lh][:]])

                def stage_gs(b, lh):
                    nc.scalar.dma_start(
                        gsb[b][lh * E:(lh + 1) * E, :, :],
                        a2a_out[b][lh].rearrange("s f r -> f s r"))

                def wo_block(b):
                    for rb in range(RQ // 128):
                        for ot in range(D // 512):
                            pw = psd_pool.tile([128, 512], f32, tag="sd")
                            for fi in range(NCORES):
                                nc.tensor.matmul(
                                    pw[:],
                                    gsb[b][:, fi, rb * 128:(rb + 1) * 128],
                                    wo_sb[:, fi, ot * 512:(ot + 1) * 512],
                                    start=(fi == 0), stop=(fi == NCORES - 1))
                            ob = ob_pool.tile([128, 512], f32, tag="ob")
                            nc.vector.tensor_add(
                                ob[:], pw[:],
                                bob_sb[:, ot * 512:(ot + 1) * 512])
                            nc.scalar.dma_start(
                                out_d[b * RQ + rb * 128:
                                      b * RQ + (rb + 1) * 128,
                                      ot * 512:(ot + 1) * 512],
                                ob[:])

                # batch 0 head 0, with the last 4 projection s-tiles
                # interleaved to keep the PE stream dense
                for st in range(4):
                    phase_a(st)
                # warmup collective, gated on late Phase-A data so its ring
                # entries cannot block the Phase-A loads; it absorbs the
                # cc-stream cold-start + inter-core skew before A2A#1
                nc.sync.dma_start(warm_d[0:1, :], qt[0:1, 3 * ST:3 * ST + 64])
                nc.gpsimd.collective_compute(
                    "AllToAll", mybir.AluOpType.bypass,
                    replica_groups=[list(range(NCORES))],
                    ins=[warm_d[:]], outs=[warm_o[:]])
                g = new_grp(0, 0)
                for t in range(NT_I):
                    attn_block(g, 0, 0, t)
                    phase_a(4 + t)
                epilogue(g, 0, 0)
                g = new_grp(0, 1)
                for t in range(NT_I):
                    attn_block(g, 0, 1, t)
                epilogue(g, 0, 1)
                g = new_grp(1, 0)
                for t in range(NT_I):
                    attn_block(g, 1, 0, t)
                epilogue(g, 1, 0)
                stage_gs(0, 0)
                stage_gs(0, 1)
                wo_block(0)
                stage_gs(1, 0)
                g = new_grp(1, 1)
                for t in range(NT_I):
                    attn_block(g, 1, 1, t)
                epilogue(g, 1, 1)
                stage_gs(1, 1)
                wo_block(1)

    _split_multi_waits(nc)
    return nc


def _get_nc():
    if _built[0] is None:
        _built[0] = _build()
    return _built[0]


def _host_inputs(x, Wq, bq, Wk, bk, Wv, bv, Wo, bo):
    xf = np.asarray(x, dtype=np.float32).reshape(BS, D)
    # [p, st, c, s] so each s-tile load is 8KB contiguous per partition
    xt = np.ascontiguousarray(
        xf.reshape(NST, ST, ND, 128).transpose(3, 0, 2, 1)).astype(bfdt)
    Wq = np.asarray(Wq, dtype=np.float32)
    Wk = np.asarray(Wk, dtype=np.float32)
    Wv = np.asarray(Wv, dtype=np.float32)
    bq = np.asarray(bq, dtype=np.float32)
    bk = np.asarray(bk, dtype=np.float32)
    bv = np.asarray(bv, dtype=np.float32)
    Wo = np.ascontiguousarray(np.asarray(Wo, dtype=np.float32)).astype(bfdt)
    bo = np.asarray(bo, dtype=np.float32)

    jj = np.arange(128, dtype=np.int64)[:, None]
    ii = np.arange(128, dtype=np.int64)[None, :]
    maskb = np.where(jj <= ii, 1.0, 0.0).astype(bfdt)
    bob = np.tile(bo[None, :], (128, 1)).astype(np.float32)
    sel32 = np.zeros((128, 4 * E), dtype=np.float32)
    for k4 in range(4):
        sel32[32 * k4, k4 * E:(k4 + 1) * E] = 1.0
    sel32 = sel32.astype(bfdt)

    in_maps = []
    for c in range(NCORES):
        hs = slice(HL * c, HL * (c + 1))
        in_maps.append({
            "xt": xt,
            "wq": np.ascontiguousarray(
                Wq[hs].transpose(1, 0, 2).reshape(D, 128)).astype(bfdt),
            "wk": np.ascontiguousarray(
                Wk[hs].transpose(1, 0, 2).reshape(D, 128)).astype(bfdt),
            "wv": np.ascontiguousarray(
                Wv[hs].transpose(1, 0, 2).reshape(D, 128)).astype(bfdt),
            "bq": np.ascontiguousarray(bq[hs].reshape(128, 1)),
            "bk": np.ascontiguousarray(bk[hs].reshape(128, 1)),
            "bv": np.ascontiguousarray(bv[hs].reshape(128, 1)),
            "wo": Wo,
            "bob": bob,
            "maskb": maskb,
            "sel32": sel32,
        })
    return in_maps


def kernel(x, Wq, bq, Wk, bk, Wv, bv, Wo, bo, _trace=False, _tmpdir=None):
    nc = _get_nc()
    in_maps = _host_inputs(x, Wq, bq, Wk, bk, Wv, bv, Wo, bo)
    res = bass_utils.run_bass_kernel_spmd(
        nc, in_maps, core_ids=list(range(NCORES)),
        trace=_trace, tmpdir=_tmpdir)
    out = np.empty((BS, D), dtype=np.float32)
    for c in range(NCORES):
        r = res.results[c]["out"]
        out[RQ * c: RQ * (c + 1)] = r[0:RQ]
        out[S + RQ * c: S + RQ * (c + 1)] = r[RQ:2 * RQ]
    kernel.last_exec_time_ns = res.exec_time_ns
    kernel.last_results = res
    return out.reshape(B, S, D)


kernel.last_exec_time_ns = None
kernel.last_results = None
